# revision 8
# baseline (speedup 1.0000x reference)
"""AttentionPooling (PMA: one learnable seed query cross-attends each ragged
segment) as a Bass/Tile kernel on 8 Trainium2 NeuronCores.

Math (exact up to fp rounding), restructured around RAW (uncentered) x:
  q = LN(seed); qh = (q @ w_q.T + b_q)/sqrt(DH)  (host)
  wq[d,h] = sum_i w_k[h*DH+i, d]*qh[h,i] (gamma folded); wv = gamma*w_v.T
  Per token t: m = mean_d x, var = mean_d x^2 - m^2, rinv = 1/sqrt(var+eps)
  v_u = x @ wv ; s_u = x @ wq      (computed from RAW x on PE)
  score = rinv*(s_u - m*cq[h]),  cq = colsum(wq)       (rank-1 mean fixup)
  e = exp(score); er = e*rinv
  num_v[b] = sum_t er*v_u ; erm[b,h] = sum_t er_h*m ; den[b,h] = sum_t e_h
  pooled[b, j in h] = (num_v[b,j] - erm[b,h]*cw[j]) / den[b,h], cw = colsum(wv)
  out = pooled @ w_o.T + bout_eff

Device dataflow per supertile of ST=8 tiles (1024 tokens), bf16:
  DMA x (bf16, 4KB/partition lines) + host-precomputed one-hot rows;
  PE: 2 transposes/tile of RAW x -> PSUM bf16; gpsimd copies PSUM->SBUF;
  PE: v-proj (256+) and score-proj (wq|mean cols) chained K=256 matmuls;
  ACT: batched PSUM->SBUF copy of v (one op per supertile) + batched Exp;
  DVE: x^2 (2x) + per-tile tensor_scalar accum (4x) for sumsq; score fixups;
  ev = er*v (2x all-SBUF); accum matmul onehot.T @ [ev|e|er*m] into PSUM acc.
  rinv via Ln/Exp batched per 4 supertiles (one ACT table set, no reloads).
Final per core: den guard, reciprocal, mean-correction of num, out-proj.
"""

import math
from contextlib import ExitStack

import ml_dtypes
import numpy as np

import concourse.bacc as bacc
import concourse.mybir as mybir
import concourse.tile as tile
from concourse.bass_utils import run_bass_kernel_spmd

P = 128          # SBUF partitions
B = 1024         # events
D = 256          # embed dim
H = 4            # heads
DH = D // H
EPS = 1e-5
NCORES = 8
BC = B // NCORES  # events per core = 128
ST = 8            # tiles per supertile
RG = 4            # supertiles per rsqrt group
F32 = mybir.dt.float32
F32R = mybir.dt.float32r
BF16 = mybir.dt.bfloat16
AF = mybir.ActivationFunctionType
OP = mybir.AluOpType

LAST_NC = None


def build_program(nt: int, cq: list):
    assert nt % ST == 0
    nst = nt // ST
    nc = bacc.Bacc("TRN2", target_bir_lowering=False, debug=False,
                   num_devices=NCORES)

    x_d = nc.dram_tensor("x", [nt * P, D], BF16, kind="ExternalInput")
    oh_d = nc.dram_tensor("oh", [nt * P, P], BF16, kind="ExternalInput")
    wvqm_d = nc.dram_tensor("wvqm", [D, D + H + 1], BF16, kind="ExternalInput")
    wot_d = nc.dram_tensor("wot", [D, D], F32R, kind="ExternalInput")
    bout_d = nc.dram_tensor("bout", [1, D], F32R, kind="ExternalInput")
    identb_d = nc.dram_tensor("identb", [P, P], BF16, kind="ExternalInput")
    ident_d = nc.dram_tensor("ident", [P, P], F32R, kind="ExternalInput")
    ones_d = nc.dram_tensor("ones", [1, P], F32R, kind="ExternalInput")
    cw_d = nc.dram_tensor("cw", [P, D], F32, kind="ExternalInput")
    out_d = nc.dram_tensor("out", [P, D], F32, kind="ExternalOutput")

    NC1 = D + H + 1   # v cols + score cols + mean col

    with tile.TileContext(nc) as tc, ExitStack() as ctx:
        singles = ctx.enter_context(tc.tile_pool(name="singles", bufs=1))
        xpool = ctx.enter_context(tc.tile_pool(name="xpool", bufs=3))
        opool = ctx.enter_context(tc.tile_pool(name="opool", bufs=10))
        tpool = ctx.enter_context(tc.tile_pool(name="tpool", bufs=2))
        vpool = ctx.enter_context(tc.tile_pool(name="vpool", bufs=10))
        rhpool = ctx.enter_context(tc.tile_pool(name="rhpool", bufs=10))
        scpool = ctx.enter_context(tc.tile_pool(name="scpool", bufs=10))
        sqpool = ctx.enter_context(tc.tile_pool(name="sqpool", bufs=2))
        gpool = ctx.enter_context(tc.tile_pool(name="gpool", bufs=3))
        epool = ctx.enter_context(tc.tile_pool(name="epool", bufs=4))
        fpool = ctx.enter_context(tc.tile_pool(name="fpool", bufs=4))
        # PSUM: v_ps 4 banks + xT_ps 2x1 bank + sc_ps 1 bank + acc 1 bank = 8
        vps_pool = ctx.enter_context(
            tc.tile_pool(name="vps", bufs=1, space="PSUM"))
        tps_pool = ctx.enter_context(
            tc.tile_pool(name="tps", bufs=2, space="PSUM"))
        sps_pool = ctx.enter_context(
            tc.tile_pool(name="sps", bufs=1, space="PSUM"))
        apool = ctx.enter_context(
            tc.tile_pool(name="apool", bufs=1, space="PSUM"))

        wvqm_sb = singles.tile([P, 2, NC1], BF16)
        nc.sync.dma_start(wvqm_sb[:, 0, :], wvqm_d[0:P, :])
        nc.sync.dma_start(wvqm_sb[:, 1, :], wvqm_d[P:2 * P, :])
        wot_sb = singles.tile([P, 2, D], F32R)
        nc.sync.dma_start(wot_sb[:, 0, :], wot_d[0:P, :])
        nc.sync.dma_start(wot_sb[:, 1, :], wot_d[P:2 * P, :])
        bout_sb = singles.tile([1, D], F32R)
        nc.sync.dma_start(bout_sb, bout_d[:])
        identb_sb = singles.tile([P, P], BF16)
        nc.sync.dma_start(identb_sb, identb_d[:])
        ident_sb = singles.tile([P, P], F32R)
        nc.sync.dma_start(ident_sb, ident_d[:])
        ones_sb = singles.tile([1, P], F32R)
        nc.sync.dma_start(ones_sb, ones_d[:])
        cw_sb = singles.tile([P, D], F32)
        nc.sync.dma_start(cw_sb, cw_d[:])
        eps_sb = singles.tile([P, 1], F32)
        nc.vector.memset(eps_sb, EPS)

        acc = apool.tile([P, D + 2 * H], F32, tag="acc")  # [ev | e | er*m]

        def phase_abc(s):
            """DMA, transposes/proj, stats for supertile s. Returns tiles
            needed later (v_sb, sc_sb, oh8, rhs8 placeholder)."""
            r0 = s * ST * P
            x8 = xpool.tile([P, ST, D], BF16, tag="x8")
            nc.sync.dma_start(
                out=x8,
                in_=x_d[r0:r0 + ST * P, :].rearrange("(p k) f -> p k f", p=P))
            oh8 = opool.tile([P, ST, P], BF16, tag="oh8")
            nc.sync.dma_start(
                out=oh8,
                in_=oh_d[r0:r0 + ST * P, :].rearrange("(p k) b -> p k b", p=P))

            # --- transposes of raw x via DMA crossbar (SBUF->SBUF) ---
            # xT[p, i, t] = x[t, i*128+p]: d-half i on dim1, token on free.
            xTs = []
            for k in range(ST):
                xT_k = tpool.tile([P, 2, P], BF16, tag=f"xT{k}")
                nc.sync.dma_start_transpose(xT_k, x8[:, k, :])
                xTs.append(xT_k)

            # --- projections: v (256 cols) and scores+mean (5 cols) ---
            v_ps = vps_pool.tile([P, ST, D], F32, tag="v_ps")
            sc_ps = sps_pool.tile([P, ST, 8], F32, tag="sc_ps")
            for k in range(ST):
                nc.tensor.matmul(v_ps[:, k, :], lhsT=xTs[k][:, 0, :],
                                 rhs=wvqm_sb[:, 0, 0:D],
                                 start=True, stop=False)
                nc.tensor.matmul(v_ps[:, k, :], lhsT=xTs[k][:, 1, :],
                                 rhs=wvqm_sb[:, 1, 0:D],
                                 start=False, stop=True)
                nc.tensor.matmul(sc_ps[:, k, 0:H + 1],
                                 lhsT=xTs[k][:, 0, :],
                                 rhs=wvqm_sb[:, 0, D:NC1],
                                 start=True, stop=False)
                nc.tensor.matmul(sc_ps[:, k, 0:H + 1],
                                 lhsT=xTs[k][:, 1, :],
                                 rhs=wvqm_sb[:, 1, D:NC1],
                                 start=False, stop=True)

            # --- batched PSUM evacuation ---
            v_sb = vpool.tile([P, ST, D], BF16, tag="v_sb")
            nc.scalar.copy(v_sb, v_ps)
            sc_sb = scpool.tile([P, ST, H + 1], F32, tag="sc_sb")
            nc.vector.tensor_copy(sc_sb, sc_ps[:, :, 0:H + 1])

            # --- stats: sumsq via x^2 (2x) + per-tile ts accum (4x) ---
            xsq = sqpool.tile([P, ST, D], BF16, tag="xsq")
            nc.vector.tensor_tensor(xsq, x8, x8, OP.mult)
            scr = sqpool.tile([P, D], BF16, tag="scr")
            ssq8 = scpool.tile([P, ST], F32, tag="ssq8")
            for k in range(ST):
                nc.vector.tensor_scalar(scr, xsq[:, k, :], 1.0 / D, None,
                                        OP.mult, OP.add,
                                        accum_out=ssq8[:, k:k + 1])
            return x8, oh8, v_sb, sc_sb, ssq8

        def phase_var(sts, tiles):
            """Group variance + rinv (batched Ln/Exp)."""
            ng = len(sts)
            var_g = gpool.tile([P, RG * ST], F32, tag="var_g")
            for i, s in enumerate(sts):
                _, _, _, sc_sb, ssq8 = tiles[i]
                msq = scpool.tile([P, ST], F32, tag="msq")
                nc.vector.tensor_tensor(msq, sc_sb[:, :, H], sc_sb[:, :, H],
                                        OP.mult)
                nc.vector.tensor_tensor(var_g[:, i * ST:(i + 1) * ST],
                                        ssq8, msq, OP.subtract)
            sl = slice(0, ng * ST)
            lnv = gpool.tile([P, RG * ST], F32, tag="lnv")
            nc.scalar.activation(lnv[:, sl], var_g[:, sl], AF.Ln, bias=eps_sb)
            nc.vector.tensor_scalar(lnv[:, sl], lnv[:, sl], -0.5, None,
                                    OP.mult)
            rinv_g = gpool.tile([P, RG * ST], F32, tag="rinv_g")
            nc.scalar.activation(rinv_g[:, sl], lnv[:, sl], AF.Exp)
            return rinv_g

        def phase_de(s, i, tiles_i, rinv_g, first, last):
            """Score fixups, exp, ev, accum for supertile s."""
            _, oh8, v_sb, sc_sb, _ = tiles_i
            rinv8 = rinv_g[:, i * ST:(i + 1) * ST]
            rhs8 = rhpool.tile([P, ST, D + 2 * H], BF16, tag="rhs8")

            # score = s_u*rinv + (m*rinv)*(-cq[h])
            sscale = epool.tile([P, ST, H], F32, tag="sscale")
            nc.vector.tensor_tensor(sscale, sc_sb[:, :, 0:H],
                                    rinv8.to_broadcast((P, ST, H)), OP.mult)
            nmr8 = epool.tile([P, ST], F32, tag="nmr8")
            nc.vector.tensor_tensor(nmr8, sc_sb[:, :, H], rinv8, OP.mult)
            scorr = epool.tile([P, ST, H], F32, tag="scorr")
            for h in range(H):
                nc.vector.tensor_scalar(scorr[:, :, h], nmr8, -cq[h], None,
                                        OP.mult)
            nc.vector.tensor_tensor(sscale, sscale, scorr, OP.add)
            nc.scalar.activation(rhs8[:, :, D:D + H], sscale, AF.Exp)

            er8 = epool.tile([P, ST, H], BF16, tag="er8")
            nc.vector.tensor_tensor(er8, rhs8[:, :, D:D + H],
                                    rinv8.to_broadcast((P, ST, H)), OP.mult)
            nc.vector.tensor_tensor(rhs8[:, :, D + H:D + 2 * H], er8,
                                    sc_sb[:, :, H].to_broadcast((P, ST, H)),
                                    OP.mult)
            nc.vector.tensor_tensor(
                out=rhs8[:, :, 0:D].rearrange("p k (h w) -> p k h w", h=H),
                in0=v_sb.rearrange("p k (h w) -> p k h w", h=H),
                in1=er8.to_broadcast((P, ST, H, DH)),
                op=OP.mult)

            for k in range(ST):
                idx = s * ST + k
                nc.tensor.matmul(acc, lhsT=oh8[:, k, :], rhs=rhs8[:, k, :],
                                 start=(idx == 0), stop=(idx == nt - 1))

        # ---- main loop: groups of RG supertiles, D/E delayed one group ----
        prev = None  # (sts, tiles, rinv_g)
        for g0 in range(0, nst, RG):
            sts = list(range(g0, min(g0 + RG, nst)))
            tiles = [phase_abc(s) for s in sts]
            rinv_g = phase_var(sts, tiles)
            if prev is not None:
                psts, ptiles, privg = prev
                for i, s in enumerate(psts):
                    phase_de(s, i, ptiles[i], privg,
                             first=(s == 0), last=(s == nst - 1))
            prev = (sts, tiles, rinv_g)
        psts, ptiles, privg = prev
        for i, s in enumerate(psts):
            phase_de(s, i, ptiles[i], privg,
                     first=(s == 0), last=(s == nst - 1))

        # ---- finalization ----
        den = acc[:, D:D + H]
        dz = fpool.tile([P, H], F32, tag="dz")
        nc.vector.tensor_scalar(dz, den, 0.0, None, OP.is_equal)
        dg = fpool.tile([P, H], F32, tag="dg")
        nc.vector.tensor_tensor(dg, den, dz, OP.add)
        rden = fpool.tile([P, H], F32, tag="rden")
        nc.vector.reciprocal(rden, dg)

        corr = fpool.tile([P, H, DH], F32, tag="corr")
        nc.vector.tensor_tensor(
            corr, acc[:, D + H:D + 2 * H].to_broadcast((P, H, DH)),
            cw_sb.rearrange("p (h w) -> p h w", h=H), OP.mult)
        nc.vector.tensor_tensor(
            corr, acc[:, 0:D].rearrange("p (h w) -> p h w", h=H),
            corr, OP.subtract)
        pooled = fpool.tile([P, D], F32R, tag="pooled")
        nc.vector.tensor_tensor(
            out=pooled.rearrange("p (h w) -> p h w", h=H),
            in0=corr, in1=rden.to_broadcast((P, H, DH)), op=OP.mult)

        pT_ps = tps_pool.tile([P, 2, P], F32R, tag="xT_ps")
        nc.tensor.transpose(pT_ps[:, 0, :], pooled[:, 0:P], ident_sb)
        nc.tensor.transpose(pT_ps[:, 1, :], pooled[:, P:2 * P], ident_sb)
        pT = fpool.tile([P, 2, P], F32R, tag="pT")
        nc.vector.tensor_copy(pT[:, 0, :], pT_ps[:, 0, :])
        nc.vector.tensor_copy(pT[:, 1, :], pT_ps[:, 1, :])

        out_ps = vps_pool.tile([P, D], F32, tag="v_ps")
        nc.tensor.matmul(out_ps, lhsT=pT[:, 0, :],
                         rhs=wot_sb[:, 0, :], start=True, stop=False)
        nc.tensor.matmul(out_ps, lhsT=pT[:, 1, :],
                         rhs=wot_sb[:, 1, :], start=False, stop=False)
        nc.tensor.matmul(out_ps, lhsT=ones_sb, rhs=bout_sb,
                         start=False, stop=True)
        out_sb = fpool.tile([P, D], F32, tag="out")
        nc.vector.tensor_copy(out_sb, out_ps)
        nc.sync.dma_start(out_d[:], out_sb)

    nc.compile()
    return nc


def _prep_weights(seed, ln_q_w, ln_q_b, ln_k_w, ln_k_b,
                  w_q, b_q, w_k, b_k, w_v, b_v, w_o, b_o):
    s = seed[0, 0].astype(np.float32)
    m = s.mean()
    v = ((s - m) ** 2).mean()
    q = (s - m) / np.sqrt(v + EPS) * ln_q_w + ln_q_b
    qh = ((q @ w_q.T + b_q) * (1.0 / np.sqrt(DH))).reshape(H, DH)
    Wq = np.einsum('hdf,hd->fh', w_k.reshape(H, DH, D), qh)      # (D, H)
    wq_t = ln_k_w[:, None] * Wq                                   # (D, H)
    wv = ln_k_w[:, None] * w_v.T                                  # (D, D)
    mean_col = np.full((D, 1), 1.0 / D, np.float32)
    WVQM = np.ascontiguousarray(
        np.concatenate([wv, wq_t, mean_col], axis=1), dtype=np.float32)
    cw = wv.sum(axis=0)                                           # (D,)
    cq = wq_t.sum(axis=0)                                         # (H,)
    cv = ln_k_b @ w_v.T + b_v                                     # (D,)
    woT = np.ascontiguousarray(w_o.T, dtype=np.float32)           # (D, D)
    bout = np.ascontiguousarray(
        (b_o + cv @ w_o.T)[None, :], dtype=np.float32)            # (1, D)
    return WVQM, woT, bout, cw, cq


def kernel(**inputs) -> np.ndarray:
    x = np.asarray(inputs["x"], dtype=np.float32)
    batch = np.asarray(inputs["batch"]).astype(np.int64)
    WVQM, woT, bout, cw, cq = _prep_weights(
        *[np.asarray(inputs[k], dtype=np.float32) for k in
          ("seed", "ln_q_w", "ln_q_b", "ln_k_w", "ln_k_b",
           "w_q", "b_q", "w_k", "b_k", "w_v", "b_v", "w_o", "b_o")])

    bounds = np.searchsorted(batch, np.arange(0, B + 1, BC))
    counts = np.diff(bounds)
    nt = max(1, math.ceil(int(counts.max()) / P))
    nt = ((nt + ST - 1) // ST) * ST
    ntok = nt * P

    ident = np.eye(P, dtype=np.float32)
    identb = ident.astype(ml_dtypes.bfloat16)
    wvqm_bf = WVQM.astype(ml_dtypes.bfloat16)
    cw_rep = np.ascontiguousarray(
        np.tile(cw[None, :], (P, 1)), dtype=np.float32)
    arangeP = np.arange(P, dtype=np.int64)

    in_maps = []
    for c in range(NCORES):
        s, e = int(bounds[c]), int(bounds[c + 1])
        n = e - s
        xc = np.zeros((ntok, D), ml_dtypes.bfloat16)
        xc[:n] = x[s:e].astype(ml_dtypes.bfloat16)
        bl = np.full((ntok,), -1, np.int64)
        bl[:n] = batch[s:e] - c * BC
        oh = (bl[:, None] == arangeP[None, :]).astype(ml_dtypes.bfloat16)
        in_maps.append({"x": xc, "oh": oh, "wvqm": wvqm_bf, "wot": woT,
                        "bout": bout, "identb": identb, "ident": ident,
                        "ones": np.ones((1, P), np.float32),
                        "cw": cw_rep})

    nc = build_program(nt, [float(v) for v in cq])
    global LAST_NC
    LAST_NC = nc
    res = run_bass_kernel_spmd(nc, in_maps, core_ids=list(range(NCORES)))
    out = np.concatenate([r["out"] for r in res.results], axis=0)
    return out.astype(np.float32)


# revision 11
# speedup vs baseline: 1.7018x; 1.7018x over previous
"""AttentionPooling (PMA: one learnable seed query cross-attends each ragged
segment) as a Bass/Tile kernel on 8 Trainium2 NeuronCores.

Math (exact up to fp rounding), restructured around RAW (uncentered) x:
  q = LN(seed); qh = (q @ w_q.T + b_q)/sqrt(DH)  (host)
  wq[d,h] = sum_i w_k[h*DH+i, d]*qh[h,i] (gamma folded); wv = gamma*w_v.T
  Per token t: m = mean_d x, var = mean_d x^2 - m^2, rinv = 1/sqrt(var+eps)
  v_u = x @ wv ; s_u = x @ wq      (computed from RAW x on PE)
  score = rinv*(s_u - m*cq[h]),  cq = colsum(wq)       (rank-1 mean fixup)
  e = exp(score); er = e*rinv
  num_v[b] = sum_t er*v_u ; erm[b,h] = sum_t er_h*m ; den[b,h] = sum_t e_h
  pooled[b, j in h] = (num_v[b,j] - erm[b,h]*cw[j]) / den[b,h], cw = colsum(wv)
  out = pooled @ w_o.T + bout_eff

Device dataflow per supertile of ST=8 tiles (1024 tokens), bf16:
  DMA x (bf16, 4KB/partition lines) + host-precomputed one-hot rows;
  PE: 2 transposes/tile of RAW x -> PSUM bf16; gpsimd copies PSUM->SBUF;
  PE: v-proj (256+) and score-proj (wq|mean cols) chained K=256 matmuls;
  ACT: batched PSUM->SBUF copy of v (one op per supertile) + batched Exp;
  DVE: x^2 (2x) + per-tile tensor_scalar accum (4x) for sumsq; score fixups;
  ev = er*v (2x all-SBUF); accum matmul onehot.T @ [ev|e|er*m] into PSUM acc.
  rinv via Ln/Exp batched per 4 supertiles (one ACT table set, no reloads).
Final per core: den guard, reciprocal, mean-correction of num, out-proj.
"""

import math
from contextlib import ExitStack

import ml_dtypes
import numpy as np

import concourse.bacc as bacc
import concourse.mybir as mybir
import concourse.tile as tile
from concourse.bass_utils import run_bass_kernel_spmd

P = 128          # SBUF partitions
B = 1024         # events
D = 256          # embed dim
H = 4            # heads
DH = D // H
EPS = 1e-5
NCORES = 8
BC = B // NCORES  # events per core = 128
ST = 8            # tiles per supertile
RG = 4            # supertiles per rsqrt group
F32 = mybir.dt.float32
F32R = mybir.dt.float32r
BF16 = mybir.dt.bfloat16
AF = mybir.ActivationFunctionType
OP = mybir.AluOpType

LAST_NC = None


def build_program(nt: int, cq: list):
    assert nt % ST == 0
    nst = nt // ST
    nc = bacc.Bacc("TRN2", target_bir_lowering=False, debug=False,
                   num_devices=NCORES)

    x_d = nc.dram_tensor("x", [nt * P, D], BF16, kind="ExternalInput")
    oh_d = nc.dram_tensor("oh", [nt * P, P], BF16, kind="ExternalInput")
    wvqm_d = nc.dram_tensor("wvqm", [D, D + H + 1], BF16, kind="ExternalInput")
    wot_d = nc.dram_tensor("wot", [D, D], F32R, kind="ExternalInput")
    bout_d = nc.dram_tensor("bout", [1, D], F32R, kind="ExternalInput")
    ident_d = nc.dram_tensor("ident", [P, P], F32R, kind="ExternalInput")
    ones_d = nc.dram_tensor("ones", [1, P], F32R, kind="ExternalInput")
    cw_d = nc.dram_tensor("cw", [P, D], F32, kind="ExternalInput")
    out_d = nc.dram_tensor("out", [P, D], F32, kind="ExternalOutput")

    NC1 = D + H + 1   # v cols + score cols + mean col

    with tile.TileContext(nc) as tc, ExitStack() as ctx:
        singles = ctx.enter_context(tc.tile_pool(name="singles", bufs=1))
        xpool = ctx.enter_context(tc.tile_pool(name="xpool", bufs=3))
        opool = ctx.enter_context(tc.tile_pool(name="opool", bufs=10))
        tpool = ctx.enter_context(tc.tile_pool(name="tpool", bufs=2))
        sqpool = ctx.enter_context(tc.tile_pool(name="sqpool", bufs=2))
        vpool = ctx.enter_context(tc.tile_pool(name="vpool", bufs=10))
        rhpool = ctx.enter_context(tc.tile_pool(name="rhpool", bufs=10))
        scpool = ctx.enter_context(tc.tile_pool(name="scpool", bufs=10))
        gpool = ctx.enter_context(tc.tile_pool(name="gpool", bufs=3))
        epool = ctx.enter_context(tc.tile_pool(name="epool", bufs=4))
        fpool = ctx.enter_context(tc.tile_pool(name="fpool", bufs=4))
        vps_pool = ctx.enter_context(
            tc.tile_pool(name="vps", bufs=1, space="PSUM"))
        sps_pool = ctx.enter_context(
            tc.tile_pool(name="sps", bufs=1, space="PSUM"))
        apool = ctx.enter_context(
            tc.tile_pool(name="apool", bufs=1, space="PSUM"))

        wvqm_sb = singles.tile([P, 2, NC1], BF16)
        nc.sync.dma_start(wvqm_sb[:, 0, :], wvqm_d[0:P, :])
        nc.sync.dma_start(wvqm_sb[:, 1, :], wvqm_d[P:2 * P, :])
        wot_sb = singles.tile([P, 2, D], F32R)
        nc.sync.dma_start(wot_sb[:, 0, :], wot_d[0:P, :])
        nc.sync.dma_start(wot_sb[:, 1, :], wot_d[P:2 * P, :])
        bout_sb = singles.tile([1, D], F32R)
        nc.sync.dma_start(bout_sb, bout_d[:])
        ident_sb = singles.tile([P, P], F32R)
        nc.sync.dma_start(ident_sb, ident_d[:])
        ones_sb = singles.tile([1, P], F32R)
        nc.sync.dma_start(ones_sb, ones_d[:])
        cw_sb = singles.tile([P, D], F32)
        nc.sync.dma_start(cw_sb, cw_d[:])
        eps_sb = singles.tile([P, 1], F32)
        nc.vector.memset(eps_sb, EPS)
        onec_sb = singles.tile([P, 1], BF16)
        nc.vector.memset(onec_sb, 1.0)

        acc = apool.tile([P, D + 2 * H], F32, tag="acc")  # [ev | e | er*m]

        def phase_abc(s):
            """DMA, transpose, proj, sumsq, evacuation for supertile s."""
            r0 = s * ST * P
            x8 = xpool.tile([P, ST, D], BF16, tag="x8")
            nc.sync.dma_start(
                out=x8,
                in_=x_d[r0:r0 + ST * P, :].rearrange("(p k) f -> p k f", p=P))
            oh8 = opool.tile([P, ST, P], BF16, tag="oh8")
            nc.sync.dma_start(
                out=oh8,
                in_=oh_d[r0:r0 + ST * P, :].rearrange("(p k) b -> p k b", p=P))

            # one batched crossbar transpose: xT[p, 2k+i, t] = x8[t, k, i*128+p]
            xT = tpool.tile([P, 2 * ST, P], BF16, tag="xT")
            nc.sync.dma_start_transpose(xT, x8)
            xsqT = sqpool.tile([P, 2 * ST, P], BF16, tag="xsqT")
            nc.vector.tensor_tensor(xsqT, xT, xT, OP.mult)

            # v projection (256 cols, bank-aligned), scores+mean (5 cols),
            # sumsq (1 col); adjacent matmuls share the same stationary xT.
            v_ps = vps_pool.tile([P, ST, D], F32, tag="v_ps")
            sc_ps = sps_pool.tile([P, ST, 8], F32, tag="sc_ps")
            sq_ps = sps_pool.tile([P, ST, 1], F32, tag="sq_ps")
            for k in range(ST):
                nc.tensor.matmul(v_ps[:, k, :], lhsT=xT[:, 2 * k, :],
                                 rhs=wvqm_sb[:, 0, 0:D], start=True,
                                 stop=False)
                nc.tensor.matmul(sc_ps[:, k, 0:H + 1], lhsT=xT[:, 2 * k, :],
                                 rhs=wvqm_sb[:, 0, D:NC1], start=True,
                                 stop=False)
                nc.tensor.matmul(v_ps[:, k, :], lhsT=xT[:, 2 * k + 1, :],
                                 rhs=wvqm_sb[:, 1, 0:D], start=False,
                                 stop=True)
                nc.tensor.matmul(sc_ps[:, k, 0:H + 1],
                                 lhsT=xT[:, 2 * k + 1, :],
                                 rhs=wvqm_sb[:, 1, D:NC1], start=False,
                                 stop=True)
                nc.tensor.matmul(sq_ps[:, k, :], lhsT=xsqT[:, 2 * k, :],
                                 rhs=onec_sb, start=True, stop=False)
                nc.tensor.matmul(sq_ps[:, k, :], lhsT=xsqT[:, 2 * k + 1, :],
                                 rhs=onec_sb, start=False, stop=True)

            v_sb = vpool.tile([P, ST, D], BF16, tag="v_sb")
            nc.scalar.copy(v_sb, v_ps)
            sc_sb = scpool.tile([P, ST, H + 1], F32, tag="sc_sb")
            nc.vector.tensor_copy(sc_sb, sc_ps[:, :, 0:H + 1])
            ssq8 = scpool.tile([P, ST], F32, tag="ssq8")
            nc.vector.tensor_copy(ssq8, sq_ps[:, :, 0])
            return oh8, v_sb, sc_sb, ssq8

        def phase_var(sts, tiles):
            """Group variance + rinv (batched Ln/Exp, one table set)."""
            ng = len(sts)
            var_g = gpool.tile([P, RG * ST], F32, tag="var_g")
            for i, s in enumerate(sts):
                _, _, sc_sb, ssq8 = tiles[i]
                msq = scpool.tile([P, ST], F32, tag="msq")
                nc.vector.tensor_tensor(msq, sc_sb[:, :, H],
                                        sc_sb[:, :, H], OP.mult)
                sl = slice(i * ST, (i + 1) * ST)
                nc.vector.tensor_scalar(var_g[:, sl], ssq8, 1.0 / D, None,
                                        OP.mult)
                nc.vector.tensor_tensor(var_g[:, sl], var_g[:, sl], msq,
                                        OP.subtract)
            sl = slice(0, ng * ST)
            lnv = gpool.tile([P, RG * ST], F32, tag="lnv")
            nc.scalar.activation(lnv[:, sl], var_g[:, sl], AF.Ln, bias=eps_sb)
            nc.vector.tensor_scalar(lnv[:, sl], lnv[:, sl], -0.5, None,
                                    OP.mult)
            rinv_g = gpool.tile([P, RG * ST], F32, tag="rinv_g")
            nc.scalar.activation(rinv_g[:, sl], lnv[:, sl], AF.Exp)
            return rinv_g

        def phase_de(s, i, tiles_i, rinv_g):
            """Score fixups, exp, ev, accum for supertile s."""
            oh8, v_sb, sc_sb, _ = tiles_i
            rinv8 = rinv_g[:, i * ST:(i + 1) * ST]
            rhs8 = rhpool.tile([P, ST, D + 2 * H], BF16, tag="rhs8")

            # score = s_u*rinv + (m*rinv)*(-cq[h])
            sscale = epool.tile([P, ST, H], F32, tag="sscale")
            nc.vector.tensor_tensor(sscale, sc_sb[:, :, 0:H],
                                    rinv8.to_broadcast((P, ST, H)), OP.mult)
            nmr8 = epool.tile([P, ST], F32, tag="nmr8")
            nc.vector.tensor_tensor(nmr8, sc_sb[:, :, H], rinv8, OP.mult)
            scorr = epool.tile([P, ST, H], F32, tag="scorr")
            for h in range(H):
                nc.gpsimd.tensor_scalar(scorr[:, :, h], nmr8, -cq[h], None,
                                        OP.mult)
            nc.vector.tensor_tensor(sscale, sscale, scorr, OP.add)
            nc.scalar.activation(rhs8[:, :, D:D + H], sscale, AF.Exp)

            er8 = epool.tile([P, ST, H], BF16, tag="er8")
            nc.gpsimd.tensor_tensor(er8, rhs8[:, :, D:D + H],
                                    rinv8.to_broadcast((P, ST, H)), OP.mult)
            nc.gpsimd.tensor_tensor(rhs8[:, :, D + H:D + 2 * H], er8,
                                    sc_sb[:, :, H].to_broadcast((P, ST, H)),
                                    OP.mult)
            nc.vector.tensor_tensor(
                out=rhs8[:, :, 0:D].rearrange("p k (h w) -> p k h w", h=H),
                in0=v_sb.rearrange("p k (h w) -> p k h w", h=H),
                in1=er8.to_broadcast((P, ST, H, DH)),
                op=OP.mult)

            for k in range(ST):
                idx = s * ST + k
                nc.tensor.matmul(acc, lhsT=oh8[:, k, :], rhs=rhs8[:, k, :],
                                 start=(idx == 0), stop=(idx == nt - 1))

        # ---- main loop: groups of RG supertiles, D/E delayed one group ----
        prev = None
        for g0 in range(0, nst, RG):
            sts = list(range(g0, min(g0 + RG, nst)))
            tiles = [phase_abc(s) for s in sts]
            rinv_g = phase_var(sts, tiles)
            if prev is not None:
                psts, ptiles, privg = prev
                for i, s in enumerate(psts):
                    phase_de(s, i, ptiles[i], privg)
            prev = (sts, tiles, rinv_g)
        psts, ptiles, privg = prev
        for i, s in enumerate(psts):
            phase_de(s, i, ptiles[i], privg)

        # ---- finalization ----
        den = acc[:, D:D + H]
        dz = fpool.tile([P, H], F32, tag="dz")
        nc.vector.tensor_scalar(dz, den, 0.0, None, OP.is_equal)
        dg = fpool.tile([P, H], F32, tag="dg")
        nc.vector.tensor_tensor(dg, den, dz, OP.add)
        rden = fpool.tile([P, H], F32, tag="rden")
        nc.vector.reciprocal(rden, dg)

        corr = fpool.tile([P, H, DH], F32, tag="corr")
        nc.vector.tensor_tensor(
            corr, acc[:, D + H:D + 2 * H].to_broadcast((P, H, DH)),
            cw_sb.rearrange("p (h w) -> p h w", h=H), OP.mult)
        nc.vector.tensor_tensor(
            corr, acc[:, 0:D].rearrange("p (h w) -> p h w", h=H),
            corr, OP.subtract)
        pooled = fpool.tile([P, D], F32R, tag="pooled")
        nc.vector.tensor_tensor(
            out=pooled.rearrange("p (h w) -> p h w", h=H),
            in0=corr, in1=rden.to_broadcast((P, H, DH)), op=OP.mult)

        pT_ps = vps_pool.tile([P, 2, P], F32R, tag="v_ps")
        nc.tensor.transpose(pT_ps[:, 0, :], pooled[:, 0:P], ident_sb)
        nc.tensor.transpose(pT_ps[:, 1, :], pooled[:, P:2 * P], ident_sb)
        pT = fpool.tile([P, 2, P], F32R, tag="pT")
        nc.vector.tensor_copy(pT[:, 0, :], pT_ps[:, 0, :])
        nc.vector.tensor_copy(pT[:, 1, :], pT_ps[:, 1, :])

        out_ps = vps_pool.tile([P, D], F32, tag="v_ps")
        nc.tensor.matmul(out_ps, lhsT=pT[:, 0, :],
                         rhs=wot_sb[:, 0, :], start=True, stop=False)
        nc.tensor.matmul(out_ps, lhsT=pT[:, 1, :],
                         rhs=wot_sb[:, 1, :], start=False, stop=False)
        nc.tensor.matmul(out_ps, lhsT=ones_sb, rhs=bout_sb,
                         start=False, stop=True)
        out_sb = fpool.tile([P, D], F32, tag="out")
        nc.vector.tensor_copy(out_sb, out_ps)
        nc.sync.dma_start(out_d[:], out_sb)

    nc.compile()
    return nc


def _prep_weights(seed, ln_q_w, ln_q_b, ln_k_w, ln_k_b,
                  w_q, b_q, w_k, b_k, w_v, b_v, w_o, b_o):
    s = seed[0, 0].astype(np.float32)
    m = s.mean()
    v = ((s - m) ** 2).mean()
    q = (s - m) / np.sqrt(v + EPS) * ln_q_w + ln_q_b
    qh = ((q @ w_q.T + b_q) * (1.0 / np.sqrt(DH))).reshape(H, DH)
    Wq = np.einsum('hdf,hd->fh', w_k.reshape(H, DH, D), qh)      # (D, H)
    wq_t = ln_k_w[:, None] * Wq                                   # (D, H)
    wv = ln_k_w[:, None] * w_v.T                                  # (D, D)
    mean_col = np.full((D, 1), 1.0 / D, np.float32)
    WVQM = np.ascontiguousarray(
        np.concatenate([wv, wq_t, mean_col], axis=1), dtype=np.float32)
    cw = wv.sum(axis=0)                                           # (D,)
    cq = wq_t.sum(axis=0)                                         # (H,)
    cv = ln_k_b @ w_v.T + b_v                                     # (D,)
    woT = np.ascontiguousarray(w_o.T, dtype=np.float32)           # (D, D)
    bout = np.ascontiguousarray(
        (b_o + cv @ w_o.T)[None, :], dtype=np.float32)            # (1, D)
    return WVQM, woT, bout, cw, cq


def kernel(**inputs) -> np.ndarray:
    x = np.asarray(inputs["x"], dtype=np.float32)
    batch = np.asarray(inputs["batch"]).astype(np.int64)
    WVQM, woT, bout, cw, cq = _prep_weights(
        *[np.asarray(inputs[k], dtype=np.float32) for k in
          ("seed", "ln_q_w", "ln_q_b", "ln_k_w", "ln_k_b",
           "w_q", "b_q", "w_k", "b_k", "w_v", "b_v", "w_o", "b_o")])

    bounds = np.searchsorted(batch, np.arange(0, B + 1, BC))
    counts = np.diff(bounds)
    nt = max(1, math.ceil(int(counts.max()) / P))
    nt = ((nt + ST - 1) // ST) * ST
    ntok = nt * P

    ident = np.eye(P, dtype=np.float32)
    wvqm_bf = WVQM.astype(ml_dtypes.bfloat16)
    cw_rep = np.ascontiguousarray(
        np.tile(cw[None, :], (P, 1)), dtype=np.float32)
    arangeP = np.arange(P, dtype=np.int64)

    in_maps = []
    for c in range(NCORES):
        s, e = int(bounds[c]), int(bounds[c + 1])
        n = e - s
        xc = np.zeros((ntok, D), ml_dtypes.bfloat16)
        xc[:n] = x[s:e].astype(ml_dtypes.bfloat16)
        bl = np.full((ntok,), -1, np.int64)
        bl[:n] = batch[s:e] - c * BC
        oh = (bl[:, None] == arangeP[None, :]).astype(ml_dtypes.bfloat16)
        in_maps.append({"x": xc, "oh": oh, "wvqm": wvqm_bf, "wot": woT,
                        "bout": bout, "ident": ident,
                        "ones": np.ones((1, P), np.float32),
                        "cw": cw_rep})

    nc = build_program(nt, [float(v) for v in cq])
    global LAST_NC
    LAST_NC = nc
    res = run_bass_kernel_spmd(nc, in_maps, core_ids=list(range(NCORES)))
    out = np.concatenate([r["out"] for r in res.results], axis=0)
    return out.astype(np.float32)


# revision 12
# speedup vs baseline: 1.8753x; 1.1020x over previous
"""AttentionPooling (PMA: one learnable seed query cross-attends each ragged
segment) as a Bass/Tile kernel on 8 Trainium2 NeuronCores.

Math (exact up to fp rounding), restructured around RAW (uncentered) x:
  q = LN(seed); qh = (q @ w_q.T + b_q)/sqrt(DH)  (host)
  wq[d,h] = sum_i w_k[h*DH+i, d]*qh[h,i] (gamma folded); wv = gamma*w_v.T
  Per token t: m = mean_d x, var = mean_d x^2 - m^2, rinv = 1/sqrt(var+eps)
  v_u = x @ wv ; s_u = x @ wq      (computed from RAW x on PE)
  score = rinv*(s_u - m*cq[h]),  cq = colsum(wq)       (rank-1 mean fixup)
  e = exp(score); er = e*rinv
  num_v[b] = sum_t er*v_u ; erm[b,h] = sum_t er_h*m ; den[b,h] = sum_t e_h
  pooled[b, j in h] = (num_v[b,j] - erm[b,h]*cw[j]) / den[b,h], cw = colsum(wv)
  out = pooled @ w_o.T + bout_eff

Device dataflow per supertile of ST=8 tiles (1024 tokens), bf16:
  DMA x (bf16, 4KB/partition lines) + host-precomputed one-hot rows;
  PE: 2 transposes/tile of RAW x -> PSUM bf16; gpsimd copies PSUM->SBUF;
  PE: v-proj (256+) and score-proj (wq|mean cols) chained K=256 matmuls;
  ACT: batched PSUM->SBUF copy of v (one op per supertile) + batched Exp;
  DVE: x^2 (2x) + per-tile tensor_scalar accum (4x) for sumsq; score fixups;
  ev = er*v (2x all-SBUF); accum matmul onehot.T @ [ev|e|er*m] into PSUM acc.
  rinv via Ln/Exp batched per 4 supertiles (one ACT table set, no reloads).
Final per core: den guard, reciprocal, mean-correction of num, out-proj.
"""

import math
from contextlib import ExitStack

import ml_dtypes
import numpy as np

import concourse.bacc as bacc
import concourse.mybir as mybir
import concourse.tile as tile
from concourse.bass_utils import run_bass_kernel_spmd

P = 128          # SBUF partitions
B = 1024         # events
D = 256          # embed dim
H = 4            # heads
DH = D // H
EPS = 1e-5
NCORES = 8
BC = B // NCORES  # events per core = 128
ST = 8            # tiles per supertile
RG = 4            # supertiles per rsqrt group
F32 = mybir.dt.float32
F32R = mybir.dt.float32r
F8 = mybir.dt.float8e4
I32 = mybir.dt.int32
BF16 = mybir.dt.bfloat16
AF = mybir.ActivationFunctionType
OP = mybir.AluOpType

LAST_NC = None


def build_program(nt: int):
    assert nt % ST == 0
    nst = nt // ST
    nc = bacc.Bacc("TRN2", target_bir_lowering=False, debug=False,
                   num_devices=NCORES)

    x_d = nc.dram_tensor("x", [nt * P, D], BF16, kind="ExternalInput")
    oh_d = nc.dram_tensor("oh", [nt * P, P], F8, kind="ExternalInput")
    wvqm_d = nc.dram_tensor("wvqm", [D, D + H + 1], BF16, kind="ExternalInput")
    wot_d = nc.dram_tensor("wot", [D, D], F32R, kind="ExternalInput")
    bout_d = nc.dram_tensor("bout", [1, D], F32R, kind="ExternalInput")
    ident_d = nc.dram_tensor("ident", [P, P], F32R, kind="ExternalInput")
    ones_d = nc.dram_tensor("ones", [1, P], F32R, kind="ExternalInput")
    out_d = nc.dram_tensor("out", [P, D], F32, kind="ExternalOutput")

    NC1 = D + H + 1   # v cols + score cols + mean col

    with tile.TileContext(nc) as tc, ExitStack() as ctx:
        singles = ctx.enter_context(tc.tile_pool(name="singles", bufs=1))
        xpool = ctx.enter_context(tc.tile_pool(name="xpool", bufs=3))
        opool = ctx.enter_context(tc.tile_pool(name="opool", bufs=10))
        tpool = ctx.enter_context(tc.tile_pool(name="tpool", bufs=2))
        sqpool = ctx.enter_context(tc.tile_pool(name="sqpool", bufs=2))
        vpool = ctx.enter_context(tc.tile_pool(name="vpool", bufs=10))
        rhpool = ctx.enter_context(tc.tile_pool(name="rhpool", bufs=10))
        scpool = ctx.enter_context(tc.tile_pool(name="scpool", bufs=10))
        gpool = ctx.enter_context(tc.tile_pool(name="gpool", bufs=3))
        epool = ctx.enter_context(tc.tile_pool(name="epool", bufs=4))
        fpool = ctx.enter_context(tc.tile_pool(name="fpool", bufs=4))
        vps_pool = ctx.enter_context(
            tc.tile_pool(name="vps", bufs=1, space="PSUM"))
        sps_pool = ctx.enter_context(
            tc.tile_pool(name="sps", bufs=1, space="PSUM"))
        apool = ctx.enter_context(
            tc.tile_pool(name="apool", bufs=1, space="PSUM"))

        wvqm_sb = singles.tile([P, 2, NC1], BF16)
        nc.sync.dma_start(wvqm_sb[:, 0, :], wvqm_d[0:P, :])
        nc.sync.dma_start(wvqm_sb[:, 1, :], wvqm_d[P:2 * P, :])
        wot_sb = singles.tile([P, 2, D], F32R)
        nc.sync.dma_start(wot_sb[:, 0, :], wot_d[0:P, :])
        nc.sync.dma_start(wot_sb[:, 1, :], wot_d[P:2 * P, :])
        bout_sb = singles.tile([1, D], F32R)
        nc.sync.dma_start(bout_sb, bout_d[:])
        ident_sb = singles.tile([P, P], F32R)
        nc.sync.dma_start(ident_sb, ident_d[:])
        ones_sb = singles.tile([1, P], F32R)
        nc.sync.dma_start(ones_sb, ones_d[:])
        onec_sb = singles.tile([P, 1], BF16)
        nc.vector.memset(onec_sb, 1.0 / D)

        acc = apool.tile([P, D + H], F32, tag="acc")  # [ev | e]

        def phase_abc(s):
            """DMA, transpose, proj, sumsq, evacuation for supertile s."""
            r0 = s * ST * P
            x8 = xpool.tile([P, ST, D], BF16, tag="x8")
            nc.sync.dma_start(
                out=x8,
                in_=x_d[r0:r0 + ST * P, :].rearrange("(p k) f -> p k f", p=P))
            oh8 = opool.tile([P, ST, P], F8, tag="oh8")
            nc.sync.dma_start(
                out=oh8,
                in_=oh_d[r0:r0 + ST * P, :].rearrange("(p k) b -> p k b", p=P))

            # one batched crossbar transpose: xT[p, 2k+i, t] = x8[t, k, i*128+p]
            xT = tpool.tile([P, 2 * ST, P], BF16, tag="xT")
            nc.sync.dma_start_transpose(xT, x8)
            xsqT = sqpool.tile([P, 2 * ST, P], BF16, tag="xsqT")
            nc.vector.tensor_tensor(xsqT, xT, xT, OP.mult)

            # v projection (256 cols, bank-aligned), scores+mean (5 cols),
            # sumsq (1 col); adjacent matmuls share the same stationary xT.
            v_ps = vps_pool.tile([P, ST, D], F32, tag="v_ps")
            sc_ps = sps_pool.tile([P, ST, 8], F32, tag="sc_ps")
            sq_ps = sps_pool.tile([P, ST, 1], F32, tag="sq_ps")
            for k in range(ST):
                nc.tensor.matmul(v_ps[:, k, :], lhsT=xT[:, 2 * k, :],
                                 rhs=wvqm_sb[:, 0, 0:D], start=True,
                                 stop=False)
                nc.tensor.matmul(sc_ps[:, k, 0:H + 1], lhsT=xT[:, 2 * k, :],
                                 rhs=wvqm_sb[:, 0, D:NC1], start=True,
                                 stop=False)
                nc.tensor.matmul(v_ps[:, k, :], lhsT=xT[:, 2 * k + 1, :],
                                 rhs=wvqm_sb[:, 1, 0:D], start=False,
                                 stop=True)
                nc.tensor.matmul(sc_ps[:, k, 0:H + 1],
                                 lhsT=xT[:, 2 * k + 1, :],
                                 rhs=wvqm_sb[:, 1, D:NC1], start=False,
                                 stop=True)
                nc.tensor.matmul(sq_ps[:, k, :], lhsT=xsqT[:, 2 * k, :],
                                 rhs=onec_sb, start=True, stop=False)
                nc.tensor.matmul(sq_ps[:, k, :], lhsT=xsqT[:, 2 * k + 1, :],
                                 rhs=onec_sb, start=False, stop=True)

            v_sb = vpool.tile([P, ST, D], BF16, tag="v_sb")
            nc.scalar.copy(v_sb, v_ps)
            sc_sb = scpool.tile([P, ST, H + 1], F32, tag="sc_sb")
            nc.vector.tensor_copy(sc_sb, sc_ps[:, :, 0:H + 1])
            ssq8 = scpool.tile([P, ST], F32, tag="ssq8")
            nc.vector.tensor_copy(ssq8, sq_ps[:, :, 0])
            return oh8, v_sb, sc_sb, ssq8

        def phase_var(sts, tiles):
            """Group variance + rinv via DVE-only Newton rsqrt."""
            ng = len(sts)
            var_g = gpool.tile([P, RG * ST], F32, tag="var_g")
            for i, s in enumerate(sts):
                _, _, sc_sb, ssq8 = tiles[i]
                msq = scpool.tile([P, ST], F32, tag="msq")
                nc.vector.tensor_tensor(msq, sc_sb[:, :, H],
                                        sc_sb[:, :, H], OP.mult)
                sl = slice(i * ST, (i + 1) * ST)
                nc.vector.tensor_tensor(var_g[:, sl], ssq8, msq, OP.subtract)
            sl = slice(0, ng * ST)
            vg = var_g[:, sl]
            nc.vector.tensor_scalar(vg, vg, EPS, None, OP.add)
            rinv_g = gpool.tile([P, RG * ST], F32, tag="rinv_g")
            y = rinv_g[:, sl]
            ti = gpool.tile([P, RG * ST], I32, tag="newt_i")
            nc.vector.tensor_scalar(ti[:, sl], vg.bitcast(I32), 1, None,
                                    OP.logical_shift_right)
            nc.vector.tensor_scalar(y.bitcast(I32), ti[:, sl], -1,
                                    0x5F3759DF, OP.mult, OP.add)
            tn = gpool.tile([P, RG * ST], F32, tag="newt_t")
            for _ in range(2):
                nc.vector.tensor_tensor(tn[:, sl], y, y, OP.mult)
                nc.vector.tensor_tensor(tn[:, sl], tn[:, sl], vg, OP.mult)
                nc.vector.tensor_scalar(tn[:, sl], tn[:, sl], -0.5, 1.5,
                                        OP.mult, OP.add)
                nc.vector.tensor_tensor(y, y, tn[:, sl], OP.mult)
            return rinv_g

        def phase_de(s, i, tiles_i, rinv_g):
            """Score scale, exp, ev, accum for supertile s."""
            oh8, v_sb, sc_sb, _ = tiles_i
            rinv8 = rinv_g[:, i * ST:(i + 1) * ST]
            rhs8 = rhpool.tile([P, ST, D + H], BF16, tag="rhs8")

            sscale = epool.tile([P, ST, H], F32, tag="sscale")
            nc.vector.tensor_tensor(sscale, sc_sb[:, :, 0:H],
                                    rinv8.to_broadcast((P, ST, H)), OP.mult)
            nc.scalar.activation(rhs8[:, :, D:D + H], sscale, AF.Exp)
            er8 = epool.tile([P, ST, H], BF16, tag="er8")
            nc.vector.tensor_tensor(er8, rhs8[:, :, D:D + H],
                                    rinv8.to_broadcast((P, ST, H)), OP.mult)
            SD = 5   # ev tiles on DVE; rest on gpsimd
            nc.vector.tensor_tensor(
                out=rhs8[:, 0:SD, 0:D].rearrange("p k (h w) -> p k h w", h=H),
                in0=v_sb[:, 0:SD, :].rearrange("p k (h w) -> p k h w", h=H),
                in1=er8[:, 0:SD, :].to_broadcast((P, SD, H, DH)),
                op=OP.mult)
            nc.gpsimd.tensor_tensor(
                out=rhs8[:, SD:ST, 0:D].rearrange("p k (h w) -> p k h w",
                                                  h=H),
                in0=v_sb[:, SD:ST, :].rearrange("p k (h w) -> p k h w", h=H),
                in1=er8[:, SD:ST, :].to_broadcast((P, ST - SD, H, DH)),
                op=OP.mult)

            for k in range(ST):
                idx = s * ST + k
                nc.tensor.matmul(acc, lhsT=oh8[:, k, :], rhs=rhs8[:, k, :],
                                 start=(idx == 0), stop=(idx == nt - 1))

        # ---- main loop: groups of RG supertiles, D/E delayed one group ----
        prev = None
        for g0 in range(0, nst, RG):
            sts = list(range(g0, min(g0 + RG, nst)))
            tiles = [phase_abc(s) for s in sts]
            rinv_g = phase_var(sts, tiles)
            if prev is not None:
                psts, ptiles, privg = prev
                for i, s in enumerate(psts):
                    phase_de(s, i, ptiles[i], privg)
            prev = (sts, tiles, rinv_g)
        psts, ptiles, privg = prev
        for i, s in enumerate(psts):
            phase_de(s, i, ptiles[i], privg)

        # ---- finalization ----
        den = acc[:, D:D + H]
        dz = fpool.tile([P, H], F32, tag="dz")
        nc.vector.tensor_scalar(dz, den, 0.0, None, OP.is_equal)
        dg = fpool.tile([P, H], F32, tag="dg")
        nc.vector.tensor_tensor(dg, den, dz, OP.add)
        rden = fpool.tile([P, H], F32, tag="rden")
        nc.vector.reciprocal(rden, dg)

        pooled = fpool.tile([P, D], F32R, tag="pooled")
        nc.vector.tensor_tensor(
            out=pooled.rearrange("p (h w) -> p h w", h=H),
            in0=acc[:, 0:D].rearrange("p (h w) -> p h w", h=H),
            in1=rden.to_broadcast((P, H, DH)), op=OP.mult)

        pT_ps = vps_pool.tile([P, 2, P], F32R, tag="v_ps")
        nc.tensor.transpose(pT_ps[:, 0, :], pooled[:, 0:P], ident_sb)
        nc.tensor.transpose(pT_ps[:, 1, :], pooled[:, P:2 * P], ident_sb)
        pT = fpool.tile([P, 2, P], F32R, tag="pT")
        nc.vector.tensor_copy(pT[:, 0, :], pT_ps[:, 0, :])
        nc.vector.tensor_copy(pT[:, 1, :], pT_ps[:, 1, :])

        out_ps = vps_pool.tile([P, D], F32, tag="v_ps")
        nc.tensor.matmul(out_ps, lhsT=pT[:, 0, :],
                         rhs=wot_sb[:, 0, :], start=True, stop=False)
        nc.tensor.matmul(out_ps, lhsT=pT[:, 1, :],
                         rhs=wot_sb[:, 1, :], start=False, stop=False)
        nc.tensor.matmul(out_ps, lhsT=ones_sb, rhs=bout_sb,
                         start=False, stop=True)
        out_sb = fpool.tile([P, D], F32, tag="out")
        nc.vector.tensor_copy(out_sb, out_ps)
        nc.sync.dma_start(out_d[:], out_sb)

    nc.compile()
    return nc


def _prep_weights(seed, ln_q_w, ln_q_b, ln_k_w, ln_k_b,
                  w_q, b_q, w_k, b_k, w_v, b_v, w_o, b_o):
    s = seed[0, 0].astype(np.float32)
    m = s.mean()
    v = ((s - m) ** 2).mean()
    q = (s - m) / np.sqrt(v + EPS) * ln_q_w + ln_q_b
    qh = ((q @ w_q.T + b_q) * (1.0 / np.sqrt(DH))).reshape(H, DH)
    Wq = np.einsum('hdf,hd->fh', w_k.reshape(H, DH, D), qh)      # (D, H)
    wq_t = ln_k_w[:, None] * Wq                                   # (D, H)
    wv = ln_k_w[:, None] * w_v.T                                  # (D, D)
    # fold mean-centering into the weights: (I - 11^T/D) w = w - 1*colsum(w)/D
    wv_c = wv - np.ones((D, 1), np.float32) * (wv.sum(axis=0) / D)[None, :]
    wq_c = wq_t - np.ones((D, 1), np.float32) * (wq_t.sum(axis=0) / D)[None, :]
    mean_col = np.full((D, 1), 1.0 / D, np.float32)
    WVQM = np.ascontiguousarray(
        np.concatenate([wv_c, wq_c, mean_col], axis=1), dtype=np.float32)
    cv = ln_k_b @ w_v.T + b_v                                     # (D,)
    woT = np.ascontiguousarray(w_o.T, dtype=np.float32)           # (D, D)
    bout = np.ascontiguousarray(
        (b_o + cv @ w_o.T)[None, :], dtype=np.float32)            # (1, D)
    return WVQM, woT, bout


def kernel(**inputs) -> np.ndarray:
    x = np.asarray(inputs["x"], dtype=np.float32)
    batch = np.asarray(inputs["batch"]).astype(np.int64)
    WVQM, woT, bout = _prep_weights(
        *[np.asarray(inputs[k], dtype=np.float32) for k in
          ("seed", "ln_q_w", "ln_q_b", "ln_k_w", "ln_k_b",
           "w_q", "b_q", "w_k", "b_k", "w_v", "b_v", "w_o", "b_o")])

    bounds = np.searchsorted(batch, np.arange(0, B + 1, BC))
    counts = np.diff(bounds)
    nt = max(1, math.ceil(int(counts.max()) / P))
    nt = ((nt + ST - 1) // ST) * ST
    ntok = nt * P

    ident = np.eye(P, dtype=np.float32)
    wvqm_bf = WVQM.astype(ml_dtypes.bfloat16)
    arangeP = np.arange(P, dtype=np.int64)

    in_maps = []
    for c in range(NCORES):
        s, e = int(bounds[c]), int(bounds[c + 1])
        n = e - s
        xc = np.zeros((ntok, D), ml_dtypes.bfloat16)
        xc[:n] = x[s:e].astype(ml_dtypes.bfloat16)
        bl = np.full((ntok,), -1, np.int64)
        bl[:n] = batch[s:e] - c * BC
        oh = (bl[:, None] == arangeP[None, :]).astype(ml_dtypes.float8_e4m3)
        in_maps.append({"x": xc, "oh": oh, "wvqm": wvqm_bf, "wot": woT,
                        "bout": bout, "ident": ident,
                        "ones": np.ones((1, P), np.float32)})

    nc = build_program(nt)
    global LAST_NC
    LAST_NC = nc
    res = run_bass_kernel_spmd(nc, in_maps, core_ids=list(range(NCORES)))
    out = np.concatenate([r["out"] for r in res.results], axis=0)
    return out.astype(np.float32)


# revision 13
# speedup vs baseline: 1.8809x; 1.0030x over previous
"""AttentionPooling (PMA: one learnable seed query cross-attends each ragged
segment) as a Bass/Tile kernel on 8 Trainium2 NeuronCores.

Math (exact up to fp rounding), restructured around RAW (uncentered) x:
  q = LN(seed); qh = (q @ w_q.T + b_q)/sqrt(DH)  (host)
  wq[d,h] = sum_i w_k[h*DH+i, d]*qh[h,i] (gamma folded); wv = gamma*w_v.T
  Per token t: m = mean_d x, var = mean_d x^2 - m^2, rinv = 1/sqrt(var+eps)
  v_u = x @ wv ; s_u = x @ wq      (computed from RAW x on PE)
  score = rinv*(s_u - m*cq[h]),  cq = colsum(wq)       (rank-1 mean fixup)
  e = exp(score); er = e*rinv
  num_v[b] = sum_t er*v_u ; erm[b,h] = sum_t er_h*m ; den[b,h] = sum_t e_h
  pooled[b, j in h] = (num_v[b,j] - erm[b,h]*cw[j]) / den[b,h], cw = colsum(wv)
  out = pooled @ w_o.T + bout_eff

Device dataflow per supertile of ST=8 tiles (1024 tokens), bf16:
  DMA x (bf16, 4KB/partition lines) + host-precomputed one-hot rows;
  PE: 2 transposes/tile of RAW x -> PSUM bf16; gpsimd copies PSUM->SBUF;
  PE: v-proj (256+) and score-proj (wq|mean cols) chained K=256 matmuls;
  ACT: batched PSUM->SBUF copy of v (one op per supertile) + batched Exp;
  DVE: x^2 (2x) + per-tile tensor_scalar accum (4x) for sumsq; score fixups;
  ev = er*v (2x all-SBUF); accum matmul onehot.T @ [ev|e|er*m] into PSUM acc.
  rinv via Ln/Exp batched per 4 supertiles (one ACT table set, no reloads).
Final per core: den guard, reciprocal, mean-correction of num, out-proj.
"""

import math
from contextlib import ExitStack

import ml_dtypes
import numpy as np

import concourse.bacc as bacc
import concourse.mybir as mybir
import concourse.tile as tile
from concourse.bass_utils import run_bass_kernel_spmd

P = 128          # SBUF partitions
B = 1024         # events
D = 256          # embed dim
H = 4            # heads
DH = D // H
EPS = 1e-5
NCORES = 8
BC = B // NCORES  # events per core = 128
ST = 8            # tiles per supertile
RG = 4            # supertiles per rsqrt group
F32 = mybir.dt.float32
F32R = mybir.dt.float32r
F8 = mybir.dt.float8e4
I32 = mybir.dt.int32
BF16 = mybir.dt.bfloat16
AF = mybir.ActivationFunctionType
OP = mybir.AluOpType

LAST_NC = None


def build_program(nt: int):
    assert nt % ST == 0
    nst = nt // ST
    nc = bacc.Bacc("TRN2", target_bir_lowering=False, debug=False,
                   num_devices=NCORES)

    x_d = nc.dram_tensor("x", [nt * P, D], BF16, kind="ExternalInput")
    oh_d = nc.dram_tensor("oh", [nt * P, P], F8, kind="ExternalInput")
    wvqm_d = nc.dram_tensor("wvqm", [D, D + H + 1], BF16, kind="ExternalInput")
    wot_d = nc.dram_tensor("wot", [D, D], F32R, kind="ExternalInput")
    bout_d = nc.dram_tensor("bout", [1, D], F32R, kind="ExternalInput")
    ident_d = nc.dram_tensor("ident", [P, P], F32R, kind="ExternalInput")
    ones_d = nc.dram_tensor("ones", [1, P], F32R, kind="ExternalInput")
    out_d = nc.dram_tensor("out", [P, D], F32, kind="ExternalOutput")

    NC1 = D + H + 1   # v cols + score cols + mean col

    with tile.TileContext(nc) as tc, ExitStack() as ctx:
        singles = ctx.enter_context(tc.tile_pool(name="singles", bufs=1))
        xpool = ctx.enter_context(tc.tile_pool(name="xpool", bufs=6))
        opool = ctx.enter_context(tc.tile_pool(name="opool", bufs=10))
        tpool = ctx.enter_context(tc.tile_pool(name="tpool", bufs=6))
        sqpool = ctx.enter_context(tc.tile_pool(name="sqpool", bufs=3))
        vpool = ctx.enter_context(tc.tile_pool(name="vpool", bufs=10))
        rhpool = ctx.enter_context(tc.tile_pool(name="rhpool", bufs=10))
        scpool = ctx.enter_context(tc.tile_pool(name="scpool", bufs=10))
        gpool = ctx.enter_context(tc.tile_pool(name="gpool", bufs=3))
        epool = ctx.enter_context(tc.tile_pool(name="epool", bufs=4))
        fpool = ctx.enter_context(tc.tile_pool(name="fpool", bufs=4))
        vps_pool = ctx.enter_context(
            tc.tile_pool(name="vps", bufs=1, space="PSUM"))
        sps_pool = ctx.enter_context(
            tc.tile_pool(name="sps", bufs=1, space="PSUM"))
        apool = ctx.enter_context(
            tc.tile_pool(name="apool", bufs=1, space="PSUM"))

        wvqm_sb = singles.tile([P, 2, NC1], BF16)
        nc.sync.dma_start(wvqm_sb[:, 0, :], wvqm_d[0:P, :])
        nc.sync.dma_start(wvqm_sb[:, 1, :], wvqm_d[P:2 * P, :])
        wot_sb = singles.tile([P, 2, D], F32R)
        nc.sync.dma_start(wot_sb[:, 0, :], wot_d[0:P, :])
        nc.sync.dma_start(wot_sb[:, 1, :], wot_d[P:2 * P, :])
        bout_sb = singles.tile([1, D], F32R)
        nc.sync.dma_start(bout_sb, bout_d[:])
        ident_sb = singles.tile([P, P], F32R)
        nc.sync.dma_start(ident_sb, ident_d[:])
        ones_sb = singles.tile([1, P], F32R)
        nc.sync.dma_start(ones_sb, ones_d[:])
        onec_sb = singles.tile([P, 1], BF16)
        nc.vector.memset(onec_sb, 1.0 / D)

        acc = apool.tile([P, D + H], F32, tag="acc")  # [ev | e]

        def phase_dma(s):
            """DMA loads + crossbar transpose submits for supertile s."""
            r0 = s * ST * P
            x8 = xpool.tile([P, ST, D], BF16, tag="x8")
            nc.sync.dma_start(
                out=x8,
                in_=x_d[r0:r0 + ST * P, :].rearrange("(p k) f -> p k f", p=P))
            oh8 = opool.tile([P, ST, P], F8, tag="oh8")
            nc.sync.dma_start(
                out=oh8,
                in_=oh_d[r0:r0 + ST * P, :].rearrange("(p k) b -> p k b", p=P))
            # one batched crossbar transpose: xT[p, 2k+i, t] = x8[t, k, i*128+p]
            xT = tpool.tile([P, 2 * ST, P], BF16, tag="xT")
            nc.sync.dma_start_transpose(xT, x8)
            return x8, oh8, xT

        def phase_compute(s, dma_s):
            """Square, proj, sumsq, PSUM evacuation for supertile s."""
            x8, oh8, xT = dma_s
            xsqT = sqpool.tile([P, 2 * ST, P], BF16, tag="xsqT")
            nc.vector.tensor_tensor(xsqT, xT, xT, OP.mult)

            # v projection (256 cols, bank-aligned), scores+mean (5 cols),
            # sumsq (1 col); adjacent matmuls share the same stationary xT.
            v_ps = vps_pool.tile([P, ST, D], F32, tag="v_ps")
            sc_ps = sps_pool.tile([P, ST, 8], F32, tag="sc_ps")
            sq_ps = sps_pool.tile([P, ST, 1], F32, tag="sq_ps")
            for k in range(ST):
                nc.tensor.matmul(v_ps[:, k, :], lhsT=xT[:, 2 * k, :],
                                 rhs=wvqm_sb[:, 0, 0:D], start=True,
                                 stop=False)
                nc.tensor.matmul(sc_ps[:, k, 0:H + 1], lhsT=xT[:, 2 * k, :],
                                 rhs=wvqm_sb[:, 0, D:NC1], start=True,
                                 stop=False)
                nc.tensor.matmul(v_ps[:, k, :], lhsT=xT[:, 2 * k + 1, :],
                                 rhs=wvqm_sb[:, 1, 0:D], start=False,
                                 stop=True)
                nc.tensor.matmul(sc_ps[:, k, 0:H + 1],
                                 lhsT=xT[:, 2 * k + 1, :],
                                 rhs=wvqm_sb[:, 1, D:NC1], start=False,
                                 stop=True)
                nc.tensor.matmul(sq_ps[:, k, :], lhsT=xsqT[:, 2 * k, :],
                                 rhs=onec_sb, start=True, stop=False)
                nc.tensor.matmul(sq_ps[:, k, :], lhsT=xsqT[:, 2 * k + 1, :],
                                 rhs=onec_sb, start=False, stop=True)

            v_sb = vpool.tile([P, ST, D], BF16, tag="v_sb")
            nc.scalar.copy(v_sb, v_ps)
            sc_sb = scpool.tile([P, ST, H + 1], F32, tag="sc_sb")
            nc.vector.tensor_copy(sc_sb, sc_ps[:, :, 0:H + 1])
            ssq8 = scpool.tile([P, ST], F32, tag="ssq8")
            nc.vector.tensor_copy(ssq8, sq_ps[:, :, 0])
            return oh8, v_sb, sc_sb, ssq8

        def phase_var(sts, tiles):
            """Group variance + rinv via DVE-only Newton rsqrt."""
            ng = len(sts)
            var_g = gpool.tile([P, RG * ST], F32, tag="var_g")
            for i, s in enumerate(sts):
                _, _, sc_sb, ssq8 = tiles[i]
                msq = scpool.tile([P, ST], F32, tag="msq")
                nc.vector.tensor_tensor(msq, sc_sb[:, :, H],
                                        sc_sb[:, :, H], OP.mult)
                sl = slice(i * ST, (i + 1) * ST)
                nc.vector.tensor_tensor(var_g[:, sl], ssq8, msq, OP.subtract)
            sl = slice(0, ng * ST)
            vg = var_g[:, sl]
            nc.vector.tensor_scalar(vg, vg, EPS, None, OP.add)
            rinv_g = gpool.tile([P, RG * ST], F32, tag="rinv_g")
            y = rinv_g[:, sl]
            ti = gpool.tile([P, RG * ST], I32, tag="newt_i")
            nc.vector.tensor_scalar(ti[:, sl], vg.bitcast(I32), 1, None,
                                    OP.logical_shift_right)
            nc.vector.tensor_scalar(y.bitcast(I32), ti[:, sl], -1,
                                    0x5F3759DF, OP.mult, OP.add)
            tn = gpool.tile([P, RG * ST], F32, tag="newt_t")
            for _ in range(2):
                nc.vector.tensor_tensor(tn[:, sl], y, y, OP.mult)
                nc.vector.tensor_tensor(tn[:, sl], tn[:, sl], vg, OP.mult)
                nc.vector.tensor_scalar(tn[:, sl], tn[:, sl], -0.5, 1.5,
                                        OP.mult, OP.add)
                nc.vector.tensor_tensor(y, y, tn[:, sl], OP.mult)
            return rinv_g

        def phase_de(s, i, tiles_i, rinv_g):
            """Score scale, exp, ev, accum for supertile s."""
            oh8, v_sb, sc_sb, _ = tiles_i
            rinv8 = rinv_g[:, i * ST:(i + 1) * ST]
            rhs8 = rhpool.tile([P, ST, D + H], BF16, tag="rhs8")

            sscale = epool.tile([P, ST, H], F32, tag="sscale")
            nc.vector.tensor_tensor(sscale, sc_sb[:, :, 0:H],
                                    rinv8.to_broadcast((P, ST, H)), OP.mult)
            nc.scalar.activation(rhs8[:, :, D:D + H], sscale, AF.Exp)
            er8 = epool.tile([P, ST, H], BF16, tag="er8")
            nc.vector.tensor_tensor(er8, rhs8[:, :, D:D + H],
                                    rinv8.to_broadcast((P, ST, H)), OP.mult)
            SD = 5   # ev tiles on DVE; rest on gpsimd
            nc.vector.tensor_tensor(
                out=rhs8[:, 0:SD, 0:D].rearrange("p k (h w) -> p k h w", h=H),
                in0=v_sb[:, 0:SD, :].rearrange("p k (h w) -> p k h w", h=H),
                in1=er8[:, 0:SD, :].to_broadcast((P, SD, H, DH)),
                op=OP.mult)
            nc.gpsimd.tensor_tensor(
                out=rhs8[:, SD:ST, 0:D].rearrange("p k (h w) -> p k h w",
                                                  h=H),
                in0=v_sb[:, SD:ST, :].rearrange("p k (h w) -> p k h w", h=H),
                in1=er8[:, SD:ST, :].to_broadcast((P, ST - SD, H, DH)),
                op=OP.mult)

            for k in range(ST):
                idx = s * ST + k
                nc.tensor.matmul(acc, lhsT=oh8[:, k, :], rhs=rhs8[:, k, :],
                                 start=(idx == 0), stop=(idx == nt - 1))

        # ---- main loop: per rgroup: DMA submits, then previous group's
        # consume phase (ready work first - avoids head-of-line blocking),
        # then this group's compute, then var/rinv last ----
        prev = None
        for g0 in range(0, nst, RG):
            sts = list(range(g0, min(g0 + RG, nst)))
            dmas = [phase_dma(s) for s in sts]
            if prev is not None:
                psts, ptiles, privg = prev
                for i, s in enumerate(psts):
                    phase_de(s, i, ptiles[i], privg)
            tiles = [phase_compute(s, dmas[i]) for i, s in enumerate(sts)]
            rinv_g = phase_var(sts, tiles)
            prev = (sts, tiles, rinv_g)
        psts, ptiles, privg = prev
        for i, s in enumerate(psts):
            phase_de(s, i, ptiles[i], privg)

        # ---- finalization ----
        den = acc[:, D:D + H]
        dz = fpool.tile([P, H], F32, tag="dz")
        nc.vector.tensor_scalar(dz, den, 0.0, None, OP.is_equal)
        dg = fpool.tile([P, H], F32, tag="dg")
        nc.vector.tensor_tensor(dg, den, dz, OP.add)
        rden = fpool.tile([P, H], F32, tag="rden")
        nc.vector.reciprocal(rden, dg)

        pooled = fpool.tile([P, D], F32R, tag="pooled")
        nc.vector.tensor_tensor(
            out=pooled.rearrange("p (h w) -> p h w", h=H),
            in0=acc[:, 0:D].rearrange("p (h w) -> p h w", h=H),
            in1=rden.to_broadcast((P, H, DH)), op=OP.mult)

        pT_ps = vps_pool.tile([P, 2, P], F32R, tag="v_ps")
        nc.tensor.transpose(pT_ps[:, 0, :], pooled[:, 0:P], ident_sb)
        nc.tensor.transpose(pT_ps[:, 1, :], pooled[:, P:2 * P], ident_sb)
        pT = fpool.tile([P, 2, P], F32R, tag="pT")
        nc.vector.tensor_copy(pT[:, 0, :], pT_ps[:, 0, :])
        nc.vector.tensor_copy(pT[:, 1, :], pT_ps[:, 1, :])

        out_ps = vps_pool.tile([P, D], F32, tag="v_ps")
        nc.tensor.matmul(out_ps, lhsT=pT[:, 0, :],
                         rhs=wot_sb[:, 0, :], start=True, stop=False)
        nc.tensor.matmul(out_ps, lhsT=pT[:, 1, :],
                         rhs=wot_sb[:, 1, :], start=False, stop=False)
        nc.tensor.matmul(out_ps, lhsT=ones_sb, rhs=bout_sb,
                         start=False, stop=True)
        out_sb = fpool.tile([P, D], F32, tag="out")
        nc.vector.tensor_copy(out_sb, out_ps)
        nc.sync.dma_start(out_d[:], out_sb)

    nc.compile()
    return nc


def _prep_weights(seed, ln_q_w, ln_q_b, ln_k_w, ln_k_b,
                  w_q, b_q, w_k, b_k, w_v, b_v, w_o, b_o):
    s = seed[0, 0].astype(np.float32)
    m = s.mean()
    v = ((s - m) ** 2).mean()
    q = (s - m) / np.sqrt(v + EPS) * ln_q_w + ln_q_b
    qh = ((q @ w_q.T + b_q) * (1.0 / np.sqrt(DH))).reshape(H, DH)
    Wq = np.einsum('hdf,hd->fh', w_k.reshape(H, DH, D), qh)      # (D, H)
    wq_t = ln_k_w[:, None] * Wq                                   # (D, H)
    wv = ln_k_w[:, None] * w_v.T                                  # (D, D)
    # fold mean-centering into the weights: (I - 11^T/D) w = w - 1*colsum(w)/D
    wv_c = wv - np.ones((D, 1), np.float32) * (wv.sum(axis=0) / D)[None, :]
    wq_c = wq_t - np.ones((D, 1), np.float32) * (wq_t.sum(axis=0) / D)[None, :]
    mean_col = np.full((D, 1), 1.0 / D, np.float32)
    WVQM = np.ascontiguousarray(
        np.concatenate([wv_c, wq_c, mean_col], axis=1), dtype=np.float32)
    cv = ln_k_b @ w_v.T + b_v                                     # (D,)
    woT = np.ascontiguousarray(w_o.T, dtype=np.float32)           # (D, D)
    bout = np.ascontiguousarray(
        (b_o + cv @ w_o.T)[None, :], dtype=np.float32)            # (1, D)
    return WVQM, woT, bout


def kernel(**inputs) -> np.ndarray:
    x = np.asarray(inputs["x"], dtype=np.float32)
    batch = np.asarray(inputs["batch"]).astype(np.int64)
    WVQM, woT, bout = _prep_weights(
        *[np.asarray(inputs[k], dtype=np.float32) for k in
          ("seed", "ln_q_w", "ln_q_b", "ln_k_w", "ln_k_b",
           "w_q", "b_q", "w_k", "b_k", "w_v", "b_v", "w_o", "b_o")])

    bounds = np.searchsorted(batch, np.arange(0, B + 1, BC))
    counts = np.diff(bounds)
    nt = max(1, math.ceil(int(counts.max()) / P))
    nt = ((nt + ST - 1) // ST) * ST
    ntok = nt * P

    ident = np.eye(P, dtype=np.float32)
    wvqm_bf = WVQM.astype(ml_dtypes.bfloat16)
    arangeP = np.arange(P, dtype=np.int64)

    in_maps = []
    for c in range(NCORES):
        s, e = int(bounds[c]), int(bounds[c + 1])
        n = e - s
        xc = np.zeros((ntok, D), ml_dtypes.bfloat16)
        xc[:n] = x[s:e].astype(ml_dtypes.bfloat16)
        bl = np.full((ntok,), -1, np.int64)
        bl[:n] = batch[s:e] - c * BC
        oh = (bl[:, None] == arangeP[None, :]).astype(ml_dtypes.float8_e4m3)
        in_maps.append({"x": xc, "oh": oh, "wvqm": wvqm_bf, "wot": woT,
                        "bout": bout, "ident": ident,
                        "ones": np.ones((1, P), np.float32)})

    nc = build_program(nt)
    global LAST_NC
    LAST_NC = nc
    res = run_bass_kernel_spmd(nc, in_maps, core_ids=list(range(NCORES)))
    out = np.concatenate([r["out"] for r in res.results], axis=0)
    return out.astype(np.float32)


# revision 20
# speedup vs baseline: 2.3276x; 1.2375x over previous
"""AttentionPooling (PMA: one learnable seed query cross-attends each ragged
segment) as a Bass/Tile kernel on 8 Trainium2 NeuronCores.

Math (exact up to fp rounding):
  Mean-centering of LN is folded into the weights on host:
  (I - 11^T/D) w = w - 1*colsum(w)/D, so the device consumes RAW x.
  Per token: var = mean(x^2) - mean(x)^2; rinv = 1/sqrt(var+eps)
  v = x @ wv_c ; score = rinv * (x @ wq_c); e = exp(score); er = e*rinv
  num[b] = sum_t er*v ; den[b,h] = sum_t e_h ; pooled = num/den
  out = pooled @ w_o.T + bout_eff

Device dataflow per supertile of ST=8 tiles (1024 tokens):
  DMA crossbar-transposes x straight from DRAM (xT[p,i,r] = x[r, i*128+p]);
  ACT squares xT; PE: per tile, chained K=256 matmuls for v (256 cols),
  scores+mean (5 cols), and sumsq (1 col vs ones/D) - LN stats come from
  the matmul engine; DVE evacuates scores+mean+ssq (tiny), computes var;
  gpsimd runs a bit-hack Newton rsqrt (no ACT tables); ACT does one batched
  Exp; DVE multiplies er into v straight out of PSUM into bf16 rhs tiles;
  PE accumulates onehot.T @ [er*v | e] into a persistent PSUM accumulator
  (host-precomputed fp8 one-hot rows, segment boundaries per core).
Final per core: den guard, reciprocal, out-projection."""

import math
from contextlib import ExitStack

import ml_dtypes
import numpy as np

import concourse.bacc as bacc
import concourse.mybir as mybir
import concourse.tile as tile
from concourse.bass_utils import run_bass_kernel_spmd

P = 128          # SBUF partitions
B = 1024         # events
D = 256          # embed dim
H = 4            # heads
DH = D // H
EPS = 1e-5
NCORES = 8
BC = B // NCORES  # events per core = 128
ST = 8            # tiles per supertile
F32 = mybir.dt.float32
F32R = mybir.dt.float32r
F8 = mybir.dt.float8e4
I32 = mybir.dt.int32
BF16 = mybir.dt.bfloat16
AF = mybir.ActivationFunctionType
OP = mybir.AluOpType

LAST_NC = None


def build_program(nt: int):
    assert nt % ST == 0
    nst = nt // ST
    nc = bacc.Bacc("TRN2", target_bir_lowering=False, debug=False,
                   num_devices=NCORES)

    x_d = nc.dram_tensor("x", [nt * P, D], BF16, kind="ExternalInput")
    oh_d = nc.dram_tensor("oh", [nt * P, P], F8, kind="ExternalInput")
    wvqm_d = nc.dram_tensor("wvqm", [D, D + H + 1], BF16, kind="ExternalInput")
    wot_d = nc.dram_tensor("wot", [D, D], F32R, kind="ExternalInput")
    bout_d = nc.dram_tensor("bout", [1, D], F32R, kind="ExternalInput")
    ident_d = nc.dram_tensor("ident", [P, P], F32R, kind="ExternalInput")
    ones_d = nc.dram_tensor("ones", [1, P], F32R, kind="ExternalInput")
    out_d = nc.dram_tensor("out", [P, D], F32, kind="ExternalOutput")

    NC1 = D + H + 1   # v cols + score cols + mean col

    with tile.TileContext(nc) as tc, ExitStack() as ctx:
        singles = ctx.enter_context(tc.tile_pool(name="singles", bufs=1))
        opool = ctx.enter_context(tc.tile_pool(name="opool", bufs=8))
        tpool = ctx.enter_context(tc.tile_pool(name="tpool", bufs=6))
        sqpool = ctx.enter_context(tc.tile_pool(name="sqpool", bufs=4))
        rhpool = ctx.enter_context(tc.tile_pool(name="rhpool", bufs=6))
        scpool = ctx.enter_context(tc.tile_pool(name="scpool", bufs=6))
        epool = ctx.enter_context(tc.tile_pool(name="epool", bufs=6))
        fpool = ctx.enter_context(tc.tile_pool(name="fpool", bufs=4))
        vps_pool = ctx.enter_context(
            tc.tile_pool(name="vps", bufs=2, space="PSUM"))
        sps_pool = ctx.enter_context(
            tc.tile_pool(name="sps", bufs=2, space="PSUM"))
        apool = ctx.enter_context(
            tc.tile_pool(name="apool", bufs=1, space="PSUM"))

        wvqm_sb = singles.tile([P, 2, NC1], BF16)
        nc.sync.dma_start(wvqm_sb[:, 0, :], wvqm_d[0:P, :])
        nc.sync.dma_start(wvqm_sb[:, 1, :], wvqm_d[P:2 * P, :])
        wot_sb = singles.tile([P, 2, D], F32R)
        nc.sync.dma_start(wot_sb[:, 0, :], wot_d[0:P, :])
        nc.sync.dma_start(wot_sb[:, 1, :], wot_d[P:2 * P, :])
        bout_sb = singles.tile([1, D], F32R)
        nc.sync.dma_start(bout_sb, bout_d[:])
        ident_sb = singles.tile([P, P], F32R)
        nc.sync.dma_start(ident_sb, ident_d[:])
        ones_sb = singles.tile([1, P], F32R)
        nc.sync.dma_start(ones_sb, ones_d[:])
        onec_sb = singles.tile([P, 1], BF16)
        nc.vector.memset(onec_sb, 1.0 / D)

        acc = apool.tile([P, D + H], F32, tag="acc")  # [er*v | e]

        def phase_dma(s):
            r0 = s * ST * P
            oh8 = opool.tile([P, ST, P], F8, tag="oh8")
            nc.sync.dma_start(
                out=oh8,
                in_=oh_d[r0:r0 + ST * P, :].rearrange("(p k) b -> p k b", p=P))
            # xT[p, i, r] = x[r0+r, i*128+p]; tile k = rows k*128..(k+1)*128
            xT = tpool.tile([P, 2, ST * P], BF16, tag="xT")
            nc.sync.dma_start_transpose(xT, x_d[r0:r0 + ST * P, :])
            return oh8, xT

        def phase_tile(s, dma_s):
            """Everything else for supertile s (scheduler orders by deps)."""
            oh8, xT = dma_s
            xsqT = sqpool.tile([P, 2, ST * P], BF16, tag="xsqT")
            nc.scalar.activation(xsqT, xT, AF.Square)

            sc_ps = sps_pool.tile([P, ST, 8], F32, tag="sc_ps")
            sq_ps = sc_ps[:, :, 5:6]
            v_halves = []
            HT = ST // 2
            for half in range(2):
                v_ps = vps_pool.tile([P, HT, D], F32, tag="v_ps")
                v_halves.append(v_ps)
                for kk in range(HT):
                    k = half * HT + kk
                    sl = slice(k * P, (k + 1) * P)
                    nc.tensor.matmul(v_ps[:, kk, :], lhsT=xT[:, 0, sl],
                                     rhs=wvqm_sb[:, 0, 0:D], start=True,
                                     stop=False)
                    nc.tensor.matmul(sc_ps[:, k, 0:H + 1], lhsT=xT[:, 0, sl],
                                     rhs=wvqm_sb[:, 0, D:NC1], start=True,
                                     stop=False)
                    nc.tensor.matmul(v_ps[:, kk, :], lhsT=xT[:, 1, sl],
                                     rhs=wvqm_sb[:, 1, 0:D], start=False,
                                     stop=True)
                    nc.tensor.matmul(sc_ps[:, k, 0:H + 1], lhsT=xT[:, 1, sl],
                                     rhs=wvqm_sb[:, 1, D:NC1], start=False,
                                     stop=True)
                    nc.tensor.matmul(sq_ps[:, k, :], lhsT=xsqT[:, 0, sl],
                                     rhs=onec_sb, start=True, stop=False)
                    nc.tensor.matmul(sq_ps[:, k, :], lhsT=xsqT[:, 1, sl],
                                     rhs=onec_sb, start=False, stop=True)

            sc_sb = scpool.tile([P, ST, H + 2], F32, tag="sc_sb")
            nc.vector.tensor_copy(sc_sb, sc_ps[:, :, 0:H + 2])

            # var = ssq/D - mean^2  (mean in col H, ssq/D in col H+1)
            msq = scpool.tile([P, ST], F32, tag="msq")
            nc.vector.tensor_tensor(msq, sc_sb[:, :, H], sc_sb[:, :, H],
                                    OP.mult)
            var8 = scpool.tile([P, ST], F32, tag="var8")
            nc.vector.tensor_tensor(var8, sc_sb[:, :, H + 1], msq,
                                    OP.subtract)
            nc.vector.tensor_scalar(var8, var8, EPS, None, OP.add)

            # rinv = rsqrt(var) via bit-hack + 2 Newton iterations (gpsimd)
            rinv8 = epool.tile([P, ST], F32, tag="rinv8")
            ti = epool.tile([P, ST], I32, tag="newt_i")
            nc.vector.tensor_scalar(ti, var8.bitcast(I32), 1, None,
                                    OP.logical_shift_right)
            nc.vector.tensor_scalar(rinv8.bitcast(I32), ti, -1,
                                    0x5F3759DF, OP.mult, OP.add)
            tn = epool.tile([P, ST], F32, tag="newt_t")
            for _ in range(2):
                nc.gpsimd.tensor_tensor(tn, rinv8, rinv8, OP.mult)
                nc.gpsimd.tensor_tensor(tn, tn, var8, OP.mult)
                nc.gpsimd.tensor_scalar(tn, tn, -0.5, 1.5, OP.mult, OP.add)
                nc.gpsimd.tensor_tensor(rinv8, rinv8, tn, OP.mult)

            # e = exp(score*rinv) -> rhs cols; er = e*rinv
            rhs8 = rhpool.tile([P, ST, D + H], BF16, tag="rhs8")
            sscale = epool.tile([P, ST, H], F32, tag="sscale")
            nc.vector.tensor_tensor(sscale, sc_sb[:, :, 0:H],
                                    rinv8.to_broadcast((P, ST, H)), OP.mult)
            nc.scalar.activation(rhs8[:, :, D:D + H], sscale, AF.Exp)
            er8 = epool.tile([P, ST, H], BF16, tag="er8")
            nc.vector.tensor_tensor(er8, rhs8[:, :, D:D + H],
                                    rinv8.to_broadcast((P, ST, H)), OP.mult)

            # er*v straight out of PSUM (fused evacuate+weight)
            for half in range(2):
                k0 = half * HT
                nc.vector.tensor_tensor(
                    out=rhs8[:, k0:k0 + HT, 0:D].rearrange(
                        "p k (h w) -> p k h w", h=H),
                    in0=v_halves[half].rearrange("p k (h w) -> p k h w", h=H),
                    in1=er8[:, k0:k0 + HT, :].to_broadcast((P, HT, H, DH)),
                    op=OP.mult)

            for k in range(ST):
                idx = s * ST + k
                nc.tensor.matmul(acc, lhsT=oh8[:, k, :], rhs=rhs8[:, k, :],
                                 start=(idx == 0), stop=(idx == nt - 1))

        dmas = {}
        LOOKAHEAD = 3
        for s in range(min(LOOKAHEAD, nst)):
            dmas[s] = phase_dma(s)
        for s in range(nst):
            if s + LOOKAHEAD < nst:
                dmas[s + LOOKAHEAD] = phase_dma(s + LOOKAHEAD)
            phase_tile(s, dmas.pop(s))

        # ---- finalization ----
        den = acc[:, D:D + H]
        dz = fpool.tile([P, H], F32, tag="dz")
        nc.vector.tensor_scalar(dz, den, 0.0, None, OP.is_equal)
        dg = fpool.tile([P, H], F32, tag="dg")
        nc.vector.tensor_tensor(dg, den, dz, OP.add)
        rden = fpool.tile([P, H], F32, tag="rden")
        nc.vector.reciprocal(rden, dg)

        pooled = fpool.tile([P, D], F32R, tag="pooled")
        nc.vector.tensor_tensor(
            out=pooled.rearrange("p (h w) -> p h w", h=H),
            in0=acc[:, 0:D].rearrange("p (h w) -> p h w", h=H),
            in1=rden.to_broadcast((P, H, DH)), op=OP.mult)

        pT_ps = vps_pool.tile([P, 2, P], F32R, tag="v_ps")
        nc.tensor.transpose(pT_ps[:, 0, :], pooled[:, 0:P], ident_sb)
        nc.tensor.transpose(pT_ps[:, 1, :], pooled[:, P:2 * P], ident_sb)
        pT = fpool.tile([P, 2, P], F32R, tag="pT")
        nc.vector.tensor_copy(pT[:, 0, :], pT_ps[:, 0, :])
        nc.vector.tensor_copy(pT[:, 1, :], pT_ps[:, 1, :])

        out_ps = vps_pool.tile([P, D], F32, tag="v_ps")
        nc.tensor.matmul(out_ps, lhsT=pT[:, 0, :],
                         rhs=wot_sb[:, 0, :], start=True, stop=False)
        nc.tensor.matmul(out_ps, lhsT=pT[:, 1, :],
                         rhs=wot_sb[:, 1, :], start=False, stop=False)
        nc.tensor.matmul(out_ps, lhsT=ones_sb, rhs=bout_sb,
                         start=False, stop=True)
        out_sb = fpool.tile([P, D], F32, tag="out")
        nc.vector.tensor_copy(out_sb, out_ps)
        nc.sync.dma_start(out_d[:], out_sb)

    nc.compile()
    return nc


def _prep_weights(seed, ln_q_w, ln_q_b, ln_k_w, ln_k_b,
                  w_q, b_q, w_k, b_k, w_v, b_v, w_o, b_o):
    s = seed[0, 0].astype(np.float32)
    m = s.mean()
    v = ((s - m) ** 2).mean()
    q = (s - m) / np.sqrt(v + EPS) * ln_q_w + ln_q_b
    qh = ((q @ w_q.T + b_q) * (1.0 / np.sqrt(DH))).reshape(H, DH)
    Wq = np.einsum('hdf,hd->fh', w_k.reshape(H, DH, D), qh)      # (D, H)
    wq_t = ln_k_w[:, None] * Wq                                   # (D, H)
    wv = ln_k_w[:, None] * w_v.T                                  # (D, D)
    # fold mean-centering into the weights: (I - 11^T/D) w = w - 1*colsum(w)/D
    wv_c = wv - np.ones((D, 1), np.float32) * (wv.sum(axis=0) / D)[None, :]
    wq_c = wq_t - np.ones((D, 1), np.float32) * (wq_t.sum(axis=0) / D)[None, :]
    mean_col = np.full((D, 1), 1.0 / D, np.float32)
    WVQM = np.ascontiguousarray(
        np.concatenate([wv_c, wq_c, mean_col], axis=1), dtype=np.float32)
    cv = ln_k_b @ w_v.T + b_v                                     # (D,)
    woT = np.ascontiguousarray(w_o.T, dtype=np.float32)           # (D, D)
    bout = np.ascontiguousarray(
        (b_o + cv @ w_o.T)[None, :], dtype=np.float32)            # (1, D)
    return WVQM, woT, bout


def kernel(**inputs) -> np.ndarray:
    x = np.asarray(inputs["x"], dtype=np.float32)
    batch = np.asarray(inputs["batch"]).astype(np.int64)
    WVQM, woT, bout = _prep_weights(
        *[np.asarray(inputs[k], dtype=np.float32) for k in
          ("seed", "ln_q_w", "ln_q_b", "ln_k_w", "ln_k_b",
           "w_q", "b_q", "w_k", "b_k", "w_v", "b_v", "w_o", "b_o")])

    bounds = np.searchsorted(batch, np.arange(0, B + 1, BC))
    counts = np.diff(bounds)
    nt = max(1, math.ceil(int(counts.max()) / P))
    nt = ((nt + ST - 1) // ST) * ST
    ntok = nt * P

    ident = np.eye(P, dtype=np.float32)
    wvqm_bf = WVQM.astype(ml_dtypes.bfloat16)
    arangeP = np.arange(P, dtype=np.int64)

    in_maps = []
    for c in range(NCORES):
        s, e = int(bounds[c]), int(bounds[c + 1])
        n = e - s
        xc = np.zeros((ntok, D), ml_dtypes.bfloat16)
        xc[:n] = x[s:e].astype(ml_dtypes.bfloat16)
        bl = np.full((ntok,), -1, np.int64)
        bl[:n] = batch[s:e] - c * BC
        # device reads oh row (p*ST+k) for token (k*128+p) of each supertile
        blr = bl.reshape(nt // ST, ST, P).transpose(0, 2, 1).reshape(-1)
        oh = (blr[:, None] == arangeP[None, :]).astype(ml_dtypes.float8_e4m3)
        in_maps.append({"x": xc, "oh": oh, "wvqm": wvqm_bf, "wot": woT,
                        "bout": bout, "ident": ident,
                        "ones": np.ones((1, P), np.float32)})

    nc = build_program(nt)
    global LAST_NC
    LAST_NC = nc
    res = run_bass_kernel_spmd(nc, in_maps, core_ids=list(range(NCORES)))
    out = np.concatenate([r["out"] for r in res.results], axis=0)
    return out.astype(np.float32)
